# revision 42
# baseline (speedup 1.0000x reference)
"""Grouped GEMM (MoE routing) Trainium2 kernel.

Strategy: tensor-parallel shard of the output N dim across 8 NeuronCores.
Every core sees all T=8192 tokens and a 512-wide slice of every expert's
weights, so per-core work is identical regardless of segment sizes and a
single SPMD program (with the segment boundaries baked in as compile-time
constants) runs on all 8 cores.

All device data is bf16 (host-cast); PSUM accumulates in fp32.  Host packs
a, w and the output into flat [128, free] layouts so every DMA is one
contiguous per-partition line (a: 16KB, w: 8KB, out: 8KB lines):

  a_pack[p, off_s + ko*Ls + t]   = a[ts+t, ko*128+p]      (per superblock s)
  w_pack[p, (e*KO + ko)*NS + n]  = b[e][j*NS+n, ko*128+p]
  o_pack[p, NB*hs + nb*Lh + t]   = out[hs+t, j*NS + nb*128 + p]  (per half h)

Tokens are processed in "superblocks" of <=1024 (one a-tile DMA each, 16KB
lines), split into <=512-token halves (one fp32 PSUM bank per half per
128-wide n block; 8 banks total).  Per-expert weights stream in 4 chunks of
8 k-slices (1MB each), prefetched up to 3 experts ahead (paced one chunk
per a-tile DMA) so expert switches never stall; experts are processed
biggest-first so small experts' weight loads ride on earlier slack.

Matmul mapping: stationary lhsT = w chunk [k=128, n=128] (bf16, FWL),
moving rhs = a tile [k=128, tok<=512], PSUM out [n=128, tok<=512],
accumulated over the 32 k-chunks.
"""

import numpy as np
import ml_dtypes

import concourse.bacc as bacc
import concourse.bass as bass
import concourse.mybir as mybir
import concourse.tile as tile
from concourse.bass_utils import run_bass_kernel_spmd

NC = 8          # NeuronCores
P = 128         # partitions
HB = 512        # max PSUM half-block (one fp32 PSUM bank)
SB = 1024       # max superblock (one a-tile DMA)
KOC = 8         # k-slices per weight/a chunk

LAST_RESULT = {}


def _plan(seg_starts, seg_ends):
    """Per active expert: split the segment into ceil(len/512) even
    <=512-token halves (minimal PSUM-block/matmul count), then pair
    consecutive halves into <=1024-token superblocks (one a-tile DMA
    each)."""
    plan = []  # list over experts of list of (ts, Ls, halves=[(off, L)])
    for s, t in zip(seg_starts, seg_ends):
        ln = t - s
        nhalf = max(1, -(-ln // HB))
        base, rem = divmod(ln, nhalf)
        hlens = [base + (1 if i < rem else 0) for i in range(nhalf)]
        hlens = [L for L in hlens if L > 0]
        sbs = []
        p = s
        for i in range(0, len(hlens), 2):
            pair = hlens[i:i + 2]
            Ls = sum(pair)
            halves = [(0, pair[0])]
            if len(pair) == 2:
                halves.append((pair[0], pair[1]))
            sbs.append((p, Ls, halves))
            p += Ls
        plan.append(sbs)
    return plan


def _build_program(T, K, NS, EA, plan):
    f32 = mybir.dt.float32
    bf16 = mybir.dt.bfloat16
    KO = K // P
    NB = NS // P
    NCHUNK = KO // KOC

    nc = bacc.Bacc(None, target_bir_lowering=False)
    apk = nc.declare_dram_parameter("apk", [P, KO * T], bf16, isOutput=False)
    wpk = nc.declare_dram_parameter("wpk", [P, EA * KO * NS], bf16, isOutput=False)
    opk = nc.declare_dram_parameter("opk", [P, NB * T], bf16, isOutput=True)

    with tile.TileContext(nc) as tc:
        with (
            tc.tile_pool(name="wpool", bufs=4) as wpool,
            tc.tile_pool(name="apool", bufs=3) as apool,
            tc.tile_pool(name="opool", bufs=4) as opool,
            tc.tile_pool(name="warm", bufs=1) as warmpool,
            tc.tile_pool(name="psum", bufs=8, space=bass.MemorySpace.PSUM) as psum_pool,
        ):
            # The PE array boots in a half-rate p-state and ramps up only
            # after ~6us of sustained activity.  Burn that in during the
            # startup DMA window with dummy matmuls on (uninitialized)
            # scratch SBUF — no data deps, so they dispatch immediately;
            # the scratch PSUM bank is reset by its first real start=True.
            wsc = warmpool.tile([P, 640], bf16, tag="wm", name="wsc")
            pwm = psum_pool.tile([P, HB], f32, tag="ps", name="warm_ps",
                                 padded_shape=[P, HB])
            # The PE sequencer boots ~8.7us in and the first real data
            # lands ~12us in: 5 warmup matmuls (~0.65us each incl. cold
            # dispatch) fill exactly that window.  (A DMA scratch-fill was
            # tried instead of memzero: it starts the warmups LATER — the
            # PE boot itself is the gate — and delays the real first data.)
            # scalar engine on purpose: engines boot staggered (scalar
            # ~6.5us, PE ~8.2us, DVE ~9.7us) and the warmups start ~0.3us
            # after the scratch is ready, so the earliest-booting engine
            # must do the fill (DVE memzero was tried: warmups slip to
            # 10.5us and real matmuls start still in the low p-state)
            nc.scalar.memzero(wsc[:, :])
            for _ in range(5):
                nc.tensor.matmul(pwm[:, :], wsc[:, :P], wsc[:, P:],
                                 start=True, stop=True)
            cur_w = {}

            def emit_w_chunk(ei, c, defer_dma=False):
                wt = wpool.tile([P, KOC * NS], bf16, tag=f"w{c}",
                                name=f"w_e{ei}_c{c}")
                lo = (ei * KO + c * KOC) * NS
                if not defer_dma:
                    nc.sync.dma_start(out=wt[:, :],
                                      in_=wpk[:, lo:lo + KOC * NS])
                cur_w[(ei, c)] = wt

            # Weight chunks are paced one per a-tile DMA (first chunk up
            # front) so the first matmul starts as early as possible and
            # weight bursts never starve the a-tile stream.  Chunk (e,c) is
            # emitted at a-tile (e, sb0, c-1) at the latest (each expert
            # pops its own backlog of <=3 chunks plus the next expert's
            # first), always in program order before its first matmul.
            from collections import deque
            pending = deque((ei, c) for ei in range(EA)
                            for c in range(NCHUNK))
            # (0,0)'s DMAs are emitted below, interleaved per-k-slice with
            # the first a-tile's, so the first matmul waits on only ~0.4MB
            emit_w_chunk(*pending.popleft(), defer_dma=True)
            for ei in range(EA):
                for si, (ts, Ls, halves) in enumerate(plan[ei]):
                    nh = len(halves)
                    ptiles = [psum_pool.tile([P, halves[h][1]], f32, tag="ps",
                                             name=f"ps_{ts}_{h}{nb}",
                                             padded_shape=[P, HB])
                              for h in range(nh) for nb in range(NB)]
                    o_tiles = [opool.tile([P, NB * halves[h][1]], bf16, tag="o",
                                          name=f"o_{ts}_{h}",
                                          padded_shape=[P, NB * HB])
                               for h in range(nh)]
                    for c in range(NCHUNK):
                        a_tile = apool.tile([P, KOC * Ls], bf16, tag="a",
                                            name=f"a_{ts}_{c}",
                                            padded_shape=[P, KOC * SB])
                        lo = KO * ts + c * KOC * Ls
                        if ei == 0 and si == 0 and c == 0:
                            # pipeline the cold start: alternate per-k-slice
                            # pieces of the first weight chunk and a-tile so
                            # matmul koi can begin as soon as piece koi
                            # lands.  This geometric piece list is tuned:
                            # strictly per-koi pieces (16 interleaved DMAs)
                            # push later a-pieces and the next chunk's tile
                            # behind queued weight lines and gap the PE
                            # ~4.8us; coarser tails gap ~1.3us less badly.
                            wt00 = cur_w[(0, 0)]
                            low = 0  # (0*KO + 0)*NS
                            for k0, k1 in ((0, 1), (1, 2), (2, 4), (4, KOC)):
                                nc.sync.dma_start(
                                    out=wt00[:, k0 * NS:k1 * NS],
                                    in_=wpk[:, low + k0 * NS:low + k1 * NS])
                                nc.sync.dma_start(
                                    out=a_tile[:, k0 * Ls:k1 * Ls],
                                    in_=apk[:, lo + k0 * Ls:lo + k1 * Ls])
                        else:
                            nc.sync.dma_start(out=a_tile[:, :],
                                              in_=apk[:, lo:lo + KOC * Ls])
                        # never run more than bufs-1 experts ahead: a
                        # further-ahead chunk would wait on its buffer at
                        # the DMA queue head, blocking this expert's own
                        # a-tile stream behind it.  Depth 3 spreads small
                        # experts' weight loads (4MB vs only a few us of
                        # compute) into earlier spare bandwidth.
                        if pending and pending[0][0] <= ei + 3:
                            emit_w_chunk(*pending.popleft())
                        w_tile = cur_w[(ei, c)]
                        if ei == 0 and si == 0 and c == 0:
                            # k-slice-outer order here: each koi's 8 matmuls
                            # (~1.6us) cover the cold-DMA arrival of the
                            # next per-koi startup piece; bank-sequential
                            # would need koi 2+ after ~0.4us and stall
                            for koi in range(KOC):
                                ko = koi
                                for h in range(nh):
                                    hoff, Lh = halves[h]
                                    for nb in range(NB):
                                        nc.tensor.matmul(
                                            ptiles[h * NB + nb][:, :],
                                            w_tile[:, koi * NS + nb * P:
                                                   koi * NS + nb * P + P],
                                            a_tile[:, koi * Ls + hoff:
                                                   koi * Ls + hoff + Lh],
                                            start=(ko == 0),
                                            stop=False,
                                        )
                        elif c < NCHUNK - 1:
                            # bank-sequential order (all 8 k-slices into one
                            # PSUM bank before moving on) measures ~1% less
                            # per-matmul overhead than interleaving banks.
                            # (Pairing the two halves per stationary weight
                            # was tried and is WORSE — self-loading matmuls
                            # reload weights regardless, and bank-alternating
                            # accumulation pipelines worse: +4us busy.)
                            for h in range(nh):
                                hoff, Lh = halves[h]
                                for nb in range(NB):
                                    for koi in range(KOC):
                                        ko = c * KOC + koi
                                        nc.tensor.matmul(
                                            ptiles[h * NB + nb][:, :],
                                            w_tile[:, koi * NS + nb * P:
                                                   koi * NS + nb * P + P],
                                            a_tile[:, koi * Ls + hoff:
                                                   koi * Ls + hoff + Lh],
                                            start=(ko == 0),
                                            stop=False,
                                        )
                        else:
                            # last k-chunk: finish one PSUM bank at a time
                            # so its copy-out overlaps the remaining banks'
                            # matmuls (shrinks the kernel tail).  For the
                            # program's final superblock, spread the copies
                            # over vector+scalar and DMA per bank so the
                            # fully-exposed tail is as short as possible.
                            last_sb = (ei == EA - 1
                                       and si == len(plan[ei]) - 1)
                            for h in range(nh):
                                hoff, Lh = halves[h]
                                hs = ts + hoff
                                for nb in range(NB):
                                    for koi in range(KOC):
                                        ko = c * KOC + koi
                                        nc.tensor.matmul(
                                            ptiles[h * NB + nb][:, :],
                                            w_tile[:, koi * NS + nb * P:
                                                   koi * NS + nb * P + P],
                                            a_tile[:, koi * Ls + hoff:
                                                   koi * Ls + hoff + Lh],
                                            start=(ko == 0),
                                            stop=(ko == KO - 1),
                                        )
                                    osl = o_tiles[h][:, nb * Lh:
                                                     nb * Lh + Lh]
                                    psl = ptiles[h * NB + nb][:, :]
                                    if last_sb:
                                        if nb % 2:
                                            nc.scalar.copy(osl, psl)
                                        else:
                                            nc.vector.tensor_copy(osl, psl)
                                        nc.sync.dma_start(
                                            out=opk[:, NB * hs + nb * Lh:
                                                    NB * hs + nb * Lh + Lh],
                                            in_=osl)
                                    else:
                                        nc.vector.tensor_copy(osl, psl)
                                if not last_sb:
                                    nc.sync.dma_start(
                                        out=opk[:, NB * hs:NB * hs + NB * Lh],
                                        in_=o_tiles[h][:, :])
    nc.compile()
    return nc


def kernel(a, b, c, seg_indptr, weight_indices, batch_size, **_):
    T, K = a.shape
    E, N, K2 = b.shape
    assert K == K2
    NS = N // NC
    KO = K // P
    NB = NS // P

    seg = np.asarray(seg_indptr).astype(np.int64)
    widx_arr = np.asarray(weight_indices).astype(np.int64)
    segs = [(int(seg[e]), int(seg[e + 1]), int(widx_arr[e]))
            for e in range(int(batch_size)) if seg[e + 1] > seg[e]]
    # biggest experts first: maximizes early bandwidth slack for weight
    # prefetch, and the final (smallest) expert minimizes the kernel tail
    segs.sort(key=lambda x: x[0] - x[1])
    seg_starts = [s for s, _, _ in segs]
    seg_ends = [t for _, t, _ in segs]
    experts = [w for _, _, w in segs]
    EA = len(segs)
    plan = _plan(seg_starts, seg_ends)

    # ---- host packing (bf16) ----
    a = np.ascontiguousarray(a, dtype=np.float32)
    at = a.T.astype(ml_dtypes.bfloat16)            # [K, T]
    at3 = np.ascontiguousarray(at.reshape(KO, P, T).transpose(1, 0, 2))
    # a_pack: per superblock s, [P, KO*Ls] chunk at offset KO*ts
    a_pack = np.empty((P, KO * T), dtype=ml_dtypes.bfloat16)
    for sbs in plan:
        for (ts, Ls, _) in sbs:
            a_pack[:, KO * ts:KO * (ts + Ls)] = \
                at3[:, :, ts:ts + Ls].reshape(P, KO * Ls)

    b16 = np.asarray(b, dtype=np.float32).astype(ml_dtypes.bfloat16)
    in_maps = []
    for j in range(NC):
        w = np.empty((P, EA * KO * NS), dtype=ml_dtypes.bfloat16)
        for ei, e in enumerate(experts):
            # b[e] is [N, K]; out = a @ b[e].T needs W^T[k, n] = b[e][n, k]
            wt = np.ascontiguousarray(b16[e][j * NS:(j + 1) * NS, :].T)
            w[:, ei * KO * NS:(ei + 1) * KO * NS] = \
                wt.reshape(KO, P, NS).transpose(1, 0, 2).reshape(P, KO * NS)
        in_maps.append({"apk": a_pack, "wpk": w})

    nc = _build_program(T, K, NS, EA, plan)

    import os
    trace = bool(int(os.environ.get("BASS_KERNEL_TRACE", "0")))
    res = run_bass_kernel_spmd(nc, in_maps, list(range(NC)), trace=trace)
    LAST_RESULT["exec_time_ns"] = res.exec_time_ns
    LAST_RESULT["results"] = res

    out = np.empty((T, N), dtype=np.float32)
    for j in range(NC):
        opk = res.results[j]["opk"]                # [P, NB*T] bf16
        for sbs in plan:
            for (ts, Ls, halves) in sbs:
                for (hoff, Lh) in halves:
                    hs = ts + hoff
                    seg_o = opk[:, NB * hs:NB * (hs + Lh)].reshape(P, NB, Lh)
                    # out[hs+t, j*NS + nb*128 + p] = seg_o[p, nb, t]
                    out[hs:hs + Lh, j * NS:(j + 1) * NS] = \
                        seg_o.transpose(2, 1, 0).reshape(Lh, NS) \
                             .astype(np.float32)
    return out
